# revision 4
# baseline (speedup 1.0000x reference)
"""DbrxAttention (GQA + RoPE + causal) on 8 Trainium2 NeuronCores.

Tensor-parallel over heads: core c owns q heads [6c, 6c+6) and kv head c.
Phase 1 interleaves, per 512-column sequence chunk, the fp8(e4m3)
DoubleRow QKV projection (k + 6 q heads, 2x PE rate, inputs pre-scaled
by 128 host-side; the 2^28 product scale is folded into the softmax
exp) with the bf16 v projection + transpose, so the 25MB bf16
hidden-state stream for v hides under fp8 compute.  Attention is
causal, bf16, scores transposed (kv on partitions, q on free dim),
software-pipelined two j-tiles ahead; the softmax denominator uses
pair-summed P tiles (vector/gpsimd adds) to halve the ones-matmul
stream.  3-way split AllToAll (per head-pair) overlaps attention;
nothing in the attention phase ever waits on a collective.  out_proj
runs as three passes of 16 contraction k-tiles (one per AllToAll
group) with partials stashed in SBUF bf16, so ~200us of tensor work on
already-arrived head groups absorbs cross-core launch skew before the
last group's data is needed.  w_out streams on the scalar-engine HWDGE
queue, prefetched during attention.

kernel(**inputs) takes the full unsharded inputs and returns the full output.
"""

import math

import numpy as np
import ml_dtypes

import concourse.bass as bass
import concourse.mybir as mybir
from concourse import bacc
import concourse.tile as tile
from concourse.bass_utils import run_bass_kernel_spmd
from concourse.masks import make_identity

BF16 = mybir.dt.bfloat16
F32 = mybir.dt.float32
F8 = mybir.dt.float8e4
NP_BF16 = ml_dtypes.bfloat16
NP_F8 = ml_dtypes.float8_e4m3
F8_SCALE = 128.0   # per-operand scale before fp8 cast
F8_CLIP = 224.0    # TRN e4m3 max normal is 240 (inf at 256)

# full-size problem config
B, S, D = 1, 2048, 6144
H, KV, HD = 48, 8, 128
R = 8  # cores


class Cfg:
    def __init__(self, S=2048, KO=48, NQ=6, R=8, DO=6144, IC=512, CH=512,
                 OT=512, GH=2, WOPRE=3):
        self.S = S          # sequence length
        self.KO = KO        # contraction k-tiles for QKV (D = KO*128)
        self.NQ = NQ        # q heads per core
        self.R = R          # cores
        self.DO = DO        # out_proj output dim
        self.IC = IC        # attention i-chunk (free dim per scores matmul)
        self.CH = CH        # QKV s-chunk (pair of CH/2 matmuls)
        self.OT = OT        # out_proj n-chunk
        self.GH = GH        # heads per AllToAll group
        self.WOPRE = WOPRE  # w_out tiles prefetched during attention
        self.D = KO * 128
        self.SB = S // R    # seq block per core after AllToAll
        self.KO2 = R * NQ   # contraction k-tiles for out_proj (H*HD = KO2*128)
        self.NT = DO // OT
        self.NG = NQ // GH  # number of A2A groups
        self.GK = R * GH    # out_proj k-tiles per A2A group
        assert S % R == 0 and S % IC == 0 and S % CH == 0 and DO % OT == 0
        assert IC % 128 == 0
        assert NQ % GH == 0
        assert self.GK * self.NG == self.KO2


def build(cfg: Cfg) -> bacc.Bacc:
    S, KO, NQ, IC, CH = cfg.S, cfg.KO, cfg.NQ, cfg.IC, cfg.CH
    NF = NQ + 1            # fp8 e-tiles per core (k + q heads)
    NCH = S // CH
    NIC = S // IC
    ND = IC // 128         # diagonal j-tiles per i-chunk
    NJ = S // 128
    SB, KO2, NT, DO, OT = cfg.SB, cfg.KO2, cfg.NT, cfg.DO, cfg.OT
    GH, NG, GK = cfg.GH, cfg.NG, cfg.GK
    CHH = CH // 2
    KOH = KO // 2          # ko half for split fp8 DMAs
    KPH = KO // 4          # fp8 DoubleRow k-pairs per ko half
    softmax_scale = (1.0 / math.sqrt(HD)) / (F8_SCALE ** 4)
    ET_K = 0               # e-tile index of k; q head h is e-tile 1+h
    nc = bacc.Bacc("TRN2", target_bir_lowering=False, debug=False,
                   num_devices=cfg.R)

    hid_d = nc.dram_tensor("hid", [NCH, 128, KO, CH], BF16,
                           kind="ExternalInput")
    hidf_d = nc.dram_tensor("hidf", [NCH, 128, KO, CH], F8,
                            kind="ExternalInput")
    wqv_d = nc.dram_tensor("wqv", [128, KO, 128], BF16,
                           kind="ExternalInput")
    wqf_d = nc.dram_tensor("wqf", [128, NF, KO, 128], F8,
                           kind="ExternalInput")
    wo_d = nc.dram_tensor("wo", [NT, 128, KO2, OT], BF16,
                          kind="ExternalInput")
    cos_d = nc.dram_tensor("cosT", [128, S], BF16, kind="ExternalInput")
    sin_d = nc.dram_tensor("sinT", [128, S], BF16, kind="ExternalInput")
    msk_d = nc.dram_tensor("masks", [128, ND, IC], BF16, kind="ExternalInput")
    out_d = nc.dram_tensor("out", [SB, DO], F32, kind="ExternalOutput")

    with (
        tile.TileContext(nc) as tc,
        tc.tile_pool(name="dram", bufs=1, space="DRAM") as dram,
        tc.tile_pool(name="big", bufs=1) as big,
    ):
        a2a_in = [dram.tile([cfg.R, GH * 128, SB], BF16,
                            name=f"a2a_in{g}") for g in range(NG)]
        a2a_out = [dram.tile([cfg.R, GH * 128, SB], BF16,
                             name=f"a2a_out{g}") for g in range(NG)]
        qkv_sb = big.tile([128, NF, S], BF16)
        ones_sb = big.tile([128, 1], BF16)
        nc.gpsimd.memset(ones_sb[:], 1.0)
        v_nat = big.tile([128, NJ, 128], BF16)

        # ---- phase 1: per chunk, fp8 DoubleRow QKV (k + q heads) with
        # fused RoPE, then bf16 v projection + transpose.  The bf16
        # hidden-state chunk for v streams during the fp8 compute of the
        # same chunk (hid buffer is single-buffered: its DMA overlaps the
        # fp8 matmuls, which only touch the fp8 copy). ----
        with (
            tc.tile_pool(name="psk", bufs=2, space="PSUM") as psk,
            tc.tile_pool(name="psT", bufs=2, space="PSUM") as psT,
            tc.tile_pool(name="ropec", bufs=1) as ropec,
            tc.tile_pool(name="ropep", bufs=2) as ropep,
            tc.tile_pool(name="wqp", bufs=1) as wqp,
            tc.tile_pool(name="hfp", bufs=2) as hfp,
            tc.tile_pool(name="wvp", bufs=1, side="right") as wvp,
            tc.tile_pool(name="hidp", bufs=1, side="right") as hidp,
            tc.tile_pool(name="vsp", bufs=2, side="right") as vsp,
        ):
            wqf_t = wqp.tile([128, NF, KO, 128], F8)
            nc.sync.dma_start(wqf_t[:, 0:1], wqf_d.ap()[:, 0:1])
            cos_sb = ropec.tile([128, S], BF16)
            sin_sb = ropec.tile([128, S], BF16)
            ident = ropec.tile([128, 128], BF16)
            nc.sync.dma_start(cos_sb[:], cos_d.ap())
            nc.sync.dma_start(sin_sb[:], sin_d.ap())
            make_identity(nc, ident[:])

            def rope_chunk(et, ci):
                sl = slice(ci * CH, (ci + 1) * CH)
                x = qkv_sb[:, et, sl]
                rot = ropep.tile([128, CH], BF16, tag="rot",
                                 name=f"rot{et}_{ci}")
                nc.scalar.copy(rot[0:64, :], x[64:128, :])
                nc.scalar.copy(rot[64:128, :], x[0:64, :])
                nc.vector.tensor_mul(rot[:], rot[:], sin_sb[:, sl])
                nc.vector.tensor_mul(x, x, cos_sb[:, sl])
                nc.vector.tensor_add(x, x, rot[:])

            wv_t = wvp.tile([128, KO, 128], BF16)
            hf_tiles = {}

            def emit_hf_load(ci):
                hfa = hfp.tile([128, KOH, CH], F8, tag="hfA",
                               name=f"hfA{ci}")
                nc.sync.dma_start(hfa[:], hidf_d.ap()[ci][:, :KOH, :])
                hfb = hfp.tile([128, KOH, CH], F8, tag="hfB",
                               name=f"hfB{ci}")
                nc.sync.dma_start(hfb[:], hidf_d.ap()[ci][:, KOH:, :])
                hf_tiles[ci] = (hfa, hfb)

            emit_hf_load(0)
            for ef in range(1, NF):
                nc.sync.dma_start(wqf_t[:, ef:ef + 1],
                                  wqf_d.ap()[:, ef:ef + 1])
            nc.sync.dma_start(wv_t[:], wqv_d.ap())

            for ci in range(NCH):
                # hid chunk for this ci streams during the fp8 matmuls
                hc = hidp.tile([128, KO, CH], BF16, tag="hc",
                               name=f"hc{ci}")
                nc.sync.dma_start(hc[:, :KOH], hid_d.ap()[ci][:, :KOH, :])
                nc.sync.dma_start(hc[:, KOH:], hid_d.ap()[ci][:, KOH:, :])
                if ci + 1 < NCH:
                    emit_hf_load(ci + 1)
                hf = hf_tiles.pop(ci)
                for ef in range(NF):
                    ps = psk.tile([128, 1024], F32, tag="ps",
                                  name=f"qk{ef}_{ci}")
                    for kp in range(KO // 2):
                        kh, kl = divmod(kp, KPH)
                        w = wqf_t[:, ef, 2 * kp:2 * kp + 2, :]
                        h_t = hf[kh]
                        st, sp = kp == 0, kp == KO // 2 - 1
                        nc.tensor.matmul(
                            ps[:, :CHH], lhsT=w,
                            rhs=h_t[:, 2 * kl:2 * kl + 2, :CHH],
                            perf_mode=mybir.MatmulPerfMode.DoubleRow,
                            start=st, stop=sp)
                        nc.tensor.matmul(
                            ps[:, 512:512 + CHH], lhsT=w,
                            rhs=h_t[:, 2 * kl:2 * kl + 2, CHH:],
                            perf_mode=mybir.MatmulPerfMode.DoubleRow,
                            start=st, stop=sp)
                    nc.vector.tensor_copy(
                        qkv_sb[:, ef, ci * CH:ci * CH + CHH],
                        ps[:, :CHH])
                    nc.vector.tensor_copy(
                        qkv_sb[:, ef, ci * CH + CHH:(ci + 1) * CH],
                        ps[:, 512:512 + CHH])
                    rope_chunk(ef, ci)

                # v projection for this chunk (bf16)
                ps = psk.tile([128, 1024], F32, tag="ps", name=f"vp{ci}")
                for ko in range(KO):
                    st, sp = ko == 0, ko == KO - 1
                    nc.tensor.matmul(
                        ps[:, :CHH], lhsT=wv_t[:, ko, :],
                        rhs=hc[:, ko, :CHH], start=st, stop=sp)
                    nc.tensor.matmul(
                        ps[:, 512:512 + CHH], lhsT=wv_t[:, ko, :],
                        rhs=hc[:, ko, CHH:], start=st, stop=sp)
                vstg = vsp.tile([128, CH], BF16, tag="vs", name=f"vs{ci}")
                nc.vector.tensor_copy(vstg[:, :CHH], ps[:, :CHH])
                nc.vector.tensor_copy(vstg[:, CHH:], ps[:, 512:512 + CHH])
                for sl in range(CH // 128):
                    st_ = ci * (CH // 128) + sl
                    pt = psT.tile([128, 128], BF16, tag="pt",
                                  name=f"tp{st_}")
                    nc.tensor.transpose(
                        pt[:], vstg[:, sl * 128:(sl + 1) * 128], ident[:])
                    nc.vector.tensor_copy(v_nat[:, st_, :], pt[:])

        # w_out pool + oT_full live across attention + out_proj; the first
        # few w_out loads are emitted before the attention instructions so
        # the scalar-engine HWDGE queue streams them during attention.
        with (
            tc.tile_pool(name="wop", bufs=5) as wop,
            tc.tile_pool(name="oTp", bufs=1) as oTp,
        ):
            oT_full = oTp.tile([128, KO2, SB], BF16)
            wo_tiles = {}

            def emit_wo_load(g, nt):
                wo_t = wop.tile([128, GK, OT], BF16, tag="wo",
                                name=f"wo{g}_{nt}")
                nc.scalar.dma_start(
                    wo_t[:], wo_d.ap()[nt][:, g * GK:(g + 1) * GK, :])
                wo_tiles[(g, nt)] = wo_t

            for nt in range(cfg.WOPRE):
                emit_wo_load(0, nt)

            # ---- phase 2+3: attention, normalize per chunk, split A2A.
            # Nothing in this phase ever waits on a collective. ----
            with (
                tc.tile_pool(name="attw", bufs=1, side="right") as attw,
                tc.tile_pool(name="pp", bufs=5, side="right") as pp,
                tc.tile_pool(name="pap", bufs=3, side="right") as pap,
                tc.tile_pool(name="rp", bufs=2, side="right") as rp,
                tc.tile_pool(name="rbp", bufs=2, side="right") as rbp,
                tc.tile_pool(name="stg", bufs=3, side="right") as stg,
                tc.tile_pool(name="pss", bufs=4, space="PSUM") as pss,
                tc.tile_pool(name="psv", bufs=2, space="PSUM") as psv,
                tc.tile_pool(name="psd", bufs=2, space="PSUM") as psd,
            ):
                msk_sb = attw.tile([128, ND, IC], BF16)
                nc.sync.dma_start(msk_sb[:], msk_d.ap())

                kT = qkv_sb[:, ET_K, :]
                for h in range(NQ):
                    qT = qkv_sb[:, 1 + h, :]
                    g, hl = divmod(h, GH)
                    for ci in range(NIC):
                        jt_max = (ci + 1) * ND
                        njp = jt_max // 2
                        pv = psv.tile([128, IC], F32, tag="pv",
                                      name=f"pv{h}_{ci}")
                        dq = psd.tile([1, IC], F32, tag="dq",
                                      name=f"dq{h}_{ci}")
                        p2s = [None] * jt_max

                        def emit_sc(jt):
                            sc = pss.tile([128, IC], F32, tag="sc",
                                          name=f"sc{h}_{ci}_{jt}")
                            nc.tensor.matmul(
                                sc[:],
                                lhsT=kT[:, jt * 128:(jt + 1) * 128],
                                rhs=qT[:, ci * IC:(ci + 1) * IC],
                                start=True, stop=True)
                            p2 = pp.tile([128, IC], BF16, tag="p",
                                         name=f"p{h}_{ci}_{jt}")
                            nc.scalar.activation(
                                p2[:], sc[:],
                                mybir.ActivationFunctionType.Exp,
                                scale=softmax_scale)
                            if jt >= ci * ND:
                                nc.vector.tensor_mul(
                                    p2[:], p2[:],
                                    msk_sb[:, jt - ci * ND, :])
                            p2s[jt] = p2

                        def emit_av(jt):
                            nc.tensor.matmul(
                                pv[:], lhsT=v_nat[:, jt, :], rhs=p2s[jt][:],
                                start=(jt == 0), stop=(jt == jt_max - 1))

                        def emit_pa(jp):
                            # pair-summed P for the softmax denominator:
                            # halves the ones-matmul stream.  Diagonal
                            # pairs ride the vector engine (after their
                            # mask muls); the rest go to gpsimd.
                            a, b = p2s[2 * jp], p2s[2 * jp + 1]
                            pa = pap.tile([128, IC], BF16, tag="pa",
                                          name=f"pa{h}_{ci}_{jp}")
                            eng = (nc.vector if 2 * jp + 1 >= ci * ND
                                   else nc.gpsimd)
                            eng.tensor_add(pa[:], a[:], b[:])
                            nc.tensor.matmul(
                                dq[:], lhsT=ones_sb[:, 0:1], rhs=pa[:],
                                start=(jp == 0), stop=(jp == njp - 1))

                        emit_sc(0)
                        emit_sc(1)
                        for jt in range(jt_max):
                            if jt + 2 < jt_max:
                                emit_sc(jt + 2)
                            emit_av(jt)
                            if jt % 2 == 1:
                                emit_pa(jt // 2)

                        # chunk epilogue: reciprocal, normalize, ship.
                        # CAST first so the pv PSUM slot frees without
                        # waiting on the reciprocal/broadcast chain.
                        o = stg.tile([128, IC], BF16, tag="o",
                                     name=f"o{h}_{ci}")
                        nc.vector.tensor_copy(o[:], pv[:])
                        r_sb = rp.tile([1, IC], F32, tag="r",
                                       name=f"r{h}_{ci}")
                        nc.vector.reciprocal_approx_fast(r_sb[:], dq[:])
                        rb = rbp.tile([128, IC], F32, tag="rb",
                                      name=f"rb{h}_{ci}")
                        nc.gpsimd.partition_broadcast(rb[:], r_sb[:])
                        nc.vector.tensor_mul(o[:], o[:], rb[:])
                        nc.sync.dma_start(
                            a2a_in[g][2 * ci:2 * ci + 2,
                                      hl * 128:(hl + 1) * 128, :]
                            .rearrange("r p s -> p r s"),
                            o[:].rearrange("p (r s) -> p r s", r=2))
                    if hl == GH - 1:
                        nc.gpsimd.collective_compute(
                            "AllToAll", mybir.AluOpType.bypass,
                            replica_groups=[list(range(cfg.R))],
                            ins=[a2a_in[g][:]], outs=[a2a_out[g][:]])

            # ---- phase 4: out_proj on this core's seq block, three
            # passes of GK=16 k-tiles (one per A2A group), partials
            # stashed in SBUF bf16.  Pass 0/1 run while the last
            # AllToAll may still be in flight on the slowest rank. ----
            with (
                tc.tile_pool(name="ph4", bufs=1) as ph4,
                tc.tile_pool(name="obp", bufs=2) as obp,
                tc.tile_pool(name="psB", bufs=8, space="PSUM") as psB,
            ):
                def emit_gather(g):
                    nc.sync.dma_start(
                        oT_full[:, g * GK:(g + 1) * GK, :]
                        .rearrange("p (r hl) s -> p r hl s", r=cfg.R),
                        a2a_out[g][:]
                        .rearrange("r (hl p) s -> p r hl s", hl=GH))

                for g in range(NG):
                    emit_gather(g)
                stash = ph4.tile([128, NT * 2, OT], BF16)

                for g in range(NG):
                    for nt in range(NT):
                        if (g, nt) not in wo_tiles:
                            emit_wo_load(g, nt)
                        wo_t = wo_tiles[(g, nt)]
                        for mi in range(SB // 128):
                            ps = psB.tile([128, OT], F32, tag="po",
                                          name=f"po{g}_{nt}_{mi}")
                            for k in range(GK):
                                ko = g * GK + k
                                nc.tensor.matmul(
                                    ps[:],
                                    lhsT=oT_full[:, ko,
                                                 mi * 128:(mi + 1) * 128],
                                    rhs=wo_t[:, k, :],
                                    start=(k == 0), stop=(k == GK - 1))
                            sl = stash[:, nt * 2 + mi, :]
                            if g == 0:
                                nc.vector.tensor_copy(sl, ps[:])
                            elif g == 1:
                                nc.vector.tensor_add(sl, sl, ps[:])
                            else:
                                ob = obp.tile([128, OT], F32, tag="ob",
                                              name=f"ob{nt}_{mi}")
                                nc.vector.tensor_add(ob[:], sl, ps[:])
                                nc.sync.dma_start(
                                    out_d.ap()[mi * 128:(mi + 1) * 128,
                                               nt * OT:(nt + 1) * OT],
                                    ob[:])

    nc.compile()
    return nc


def make_masks(cfg: Cfg) -> np.ndarray:
    ND = cfg.IC // 128
    jj = np.arange(128)[:, None, None]
    rr = np.arange(ND)[None, :, None]
    ii = np.arange(cfg.IC)[None, None, :]
    return (jj + 128 * rr <= ii).astype(NP_BF16)


def _to_f8(x):
    return np.ascontiguousarray(
        np.clip(x * F8_SCALE, -F8_CLIP, F8_CLIP)).astype(NP_F8)


def shard_inputs(cfg: Cfg, hidden_states, cos, sin, w_qkv, w_out,
                 n_heads, n_kv):
    """Build per-core input maps (host-side shard + cast + layout)."""
    S, KO, NQ, R = cfg.S, cfg.KO, cfg.NQ, cfg.R
    D = cfg.D
    NCH, CH = S // cfg.CH, cfg.CH
    NF = NQ + 1
    hid_T = np.ascontiguousarray(hidden_states.reshape(S, D).T)  # [D, S]
    # [NCH, 128, KO, CH]
    hid_l = (hid_T.reshape(KO, 128, NCH, CH).transpose(2, 1, 0, 3)
             .astype(NP_BF16))
    hid_l = np.ascontiguousarray(hid_l)
    hidf_l = np.ascontiguousarray(
        _to_f8(hid_T).reshape(KO, 128, NCH, CH).transpose(2, 1, 0, 3))
    NT, OT, KO2 = cfg.NT, cfg.OT, cfg.KO2
    # reorder w_out rows so contraction tile ko2 = g*(R*GH) + r*GH + hl
    # maps to global head r*NQ + g*GH + hl
    NG, GH = cfg.NG, cfg.GH
    w_re = (w_out.reshape(R, NG, GH, 128, cfg.DO).transpose(1, 0, 2, 3, 4)
            .reshape(KO2 * 128, cfg.DO))
    wo_l = (w_re.reshape(KO2, 128, NT, OT).transpose(2, 1, 0, 3)
            .astype(NP_BF16))
    wo_l = np.ascontiguousarray(wo_l)
    cos_T = cos.T.astype(NP_BF16)  # [HD, S]
    sin_T = sin.T
    sinS = np.concatenate([-sin_T[:64], sin_T[64:]], axis=0).astype(NP_BF16)
    masks = make_masks(cfg)

    in_maps = []
    for c in range(R):
        qs = c * NQ * 128
        # fp8 e-tile order: k, q0..q5
        wf = np.concatenate([
            w_qkv[:, n_heads * HD + c * 128: n_heads * HD + (c + 1) * 128],
            w_qkv[:, qs:qs + NQ * 128],
        ], axis=1)  # [D, NF*128]
        wqf_l = (_to_f8(wf).reshape(KO, 128, NF, 128)
                 .transpose(1, 2, 0, 3))  # [128, NF, KO, 128]
        wv = w_qkv[:, (n_heads + n_kv) * HD + c * 128:
                   (n_heads + n_kv) * HD + (c + 1) * 128]  # [D, 128]
        wqv_l = (wv.reshape(KO, 128, 128).transpose(1, 0, 2)
                 .astype(NP_BF16))
        in_maps.append({
            "hid": hid_l, "hidf": hidf_l,
            "wqv": np.ascontiguousarray(wqv_l),
            "wqf": np.ascontiguousarray(wqf_l),
            "wo": wo_l,
            "cosT": cos_T, "sinT": sinS, "masks": masks,
        })
    return in_maps


_cached = {}


def _get_nc(cfg: Cfg):
    key = tuple(sorted(cfg.__dict__.items()))
    if key not in _cached:
        _cached[key] = build(cfg)
    return _cached[key]


def run(cfg: Cfg, in_maps, **kwargs):
    nc = _get_nc(cfg)
    res = run_bass_kernel_spmd(nc, in_maps, core_ids=list(range(cfg.R)),
                               **kwargs)
    out = np.concatenate([res.results[c]["out"] for c in range(cfg.R)],
                         axis=0)
    return out, res


def kernel(hidden_states, cos, sin, w_qkv, w_out):
    cfg = Cfg()
    hidden_states = np.asarray(hidden_states, dtype=np.float32)
    cos = np.asarray(cos, dtype=np.float32)
    sin = np.asarray(sin, dtype=np.float32)
    w_qkv = np.asarray(w_qkv, dtype=np.float32)
    w_out = np.asarray(w_out, dtype=np.float32)
    in_maps = shard_inputs(cfg, hidden_states, cos, sin, w_qkv, w_out, H, KV)
    out, _ = run(cfg, in_maps)
    return out.reshape(B, S, D).astype(np.float32)


# revision 5
# speedup vs baseline: 1.3724x; 1.3724x over previous
"""DbrxAttention (GQA + RoPE + causal) on 8 Trainium2 NeuronCores.

Tensor-parallel over heads: core c owns q heads [6c, 6c+6) and kv head c.
Phase 1 interleaves, per 512-column sequence chunk, the fp8(e4m3)
DoubleRow QKV projection (k + 6 q heads, 2x PE rate, inputs pre-scaled
by 128 host-side; the 2^28 product scale is folded into the softmax
exp) with the bf16 v projection + transpose, so the 25MB bf16
hidden-state stream for v hides under fp8 compute.  Attention is
causal, bf16, scores transposed (kv on partitions, q on free dim),
software-pipelined two j-tiles ahead; the softmax denominator uses
pair-summed P tiles (vector/gpsimd adds) to halve the ones-matmul
stream.  3-way split AllToAll (per head-pair) overlaps attention;
nothing in the attention phase ever waits on a collective.  out_proj
runs as three passes of 16 contraction k-tiles (one per AllToAll
group) with partials stashed in SBUF bf16, so ~200us of tensor work on
already-arrived head groups absorbs cross-core launch skew before the
last group's data is needed.  w_out streams on the scalar-engine HWDGE
queue, prefetched during attention.

kernel(**inputs) takes the full unsharded inputs and returns the full output.
"""

import math

import numpy as np
import ml_dtypes

import concourse.bass as bass
import concourse.mybir as mybir
from concourse import bacc
import concourse.tile as tile
from concourse.bass_utils import run_bass_kernel_spmd
from concourse.masks import make_identity

BF16 = mybir.dt.bfloat16
F32 = mybir.dt.float32
F8 = mybir.dt.float8e4
NP_BF16 = ml_dtypes.bfloat16
NP_F8 = ml_dtypes.float8_e4m3
F8_SCALE = 128.0   # per-operand scale before fp8 cast
F8_CLIP = 224.0    # TRN e4m3 max normal is 240 (inf at 256)

# full-size problem config
B, S, D = 1, 2048, 6144
H, KV, HD = 48, 8, 128
R = 8  # cores


class Cfg:
    def __init__(self, S=2048, KO=48, NQ=6, R=8, DO=6144, IC=512, CH=512,
                 OT=512, GH=2, WOPRE=3):
        self.S = S          # sequence length
        self.KO = KO        # contraction k-tiles for QKV (D = KO*128)
        self.NQ = NQ        # q heads per core
        self.R = R          # cores
        self.DO = DO        # out_proj output dim
        self.IC = IC        # attention i-chunk (free dim per scores matmul)
        self.CH = CH        # QKV s-chunk (pair of CH/2 matmuls)
        self.OT = OT        # out_proj n-chunk
        self.GH = GH        # heads per AllToAll group
        self.WOPRE = WOPRE  # w_out tiles prefetched during attention
        self.D = KO * 128
        self.SB = S // R    # seq block per core after AllToAll
        self.KO2 = R * NQ   # contraction k-tiles for out_proj (H*HD = KO2*128)
        self.NT = DO // OT
        self.NG = NQ // GH  # number of A2A groups
        self.GK = R * GH    # out_proj k-tiles per A2A group
        assert S % R == 0 and S % IC == 0 and S % CH == 0 and DO % OT == 0
        assert IC % 128 == 0
        assert NQ % GH == 0
        assert self.GK * self.NG == self.KO2


def build(cfg: Cfg) -> bacc.Bacc:
    S, KO, NQ, IC, CH = cfg.S, cfg.KO, cfg.NQ, cfg.IC, cfg.CH
    NF = NQ + 1            # fp8 e-tiles per core (k + q heads)
    NCH = S // CH
    NIC = S // IC
    ND = IC // 128         # diagonal j-tiles per i-chunk
    NJ = S // 128
    SB, KO2, NT, DO, OT = cfg.SB, cfg.KO2, cfg.NT, cfg.DO, cfg.OT
    GH, NG, GK = cfg.GH, cfg.NG, cfg.GK
    CHH = CH // 2
    KOH = KO // 2          # ko half for split fp8 DMAs
    KPH = KO // 4          # fp8 DoubleRow k-pairs per ko half
    softmax_scale = (1.0 / math.sqrt(HD)) / (F8_SCALE ** 4)
    ET_K = 0               # e-tile index of k; q head h is e-tile 1+h
    nc = bacc.Bacc("TRN2", target_bir_lowering=False, debug=False,
                   num_devices=cfg.R)

    hid_d = nc.dram_tensor("hid", [NCH, 128, KO, CH], BF16,
                           kind="ExternalInput")
    hidf_d = nc.dram_tensor("hidf", [NCH, 128, KO, CH], F8,
                            kind="ExternalInput")
    wqv_d = nc.dram_tensor("wqv", [128, KO, 128], BF16,
                           kind="ExternalInput")
    wqf_d = nc.dram_tensor("wqf", [128, NF, KO, 128], F8,
                           kind="ExternalInput")
    wo_d = nc.dram_tensor("wo", [NT, 128, KO2, OT], BF16,
                          kind="ExternalInput")
    cos_d = nc.dram_tensor("cosT", [128, S], BF16, kind="ExternalInput")
    sin_d = nc.dram_tensor("sinT", [128, S], BF16, kind="ExternalInput")
    msk_d = nc.dram_tensor("masks", [128, ND, IC], BF16, kind="ExternalInput")
    out_d = nc.dram_tensor("out", [SB, DO], F32, kind="ExternalOutput")

    with (
        tile.TileContext(nc) as tc,
        tc.tile_pool(name="dram", bufs=1, space="DRAM") as dram,
        tc.tile_pool(name="big", bufs=1) as big,
    ):
        a2a_in = [dram.tile([cfg.R, GH * 128, SB], BF16,
                            name=f"a2a_in{g}") for g in range(NG)]
        a2a_out = [dram.tile([cfg.R, GH * 128, SB], BF16,
                             name=f"a2a_out{g}") for g in range(NG)]
        qkv_sb = big.tile([128, NF, S], BF16)
        ones_sb = big.tile([128, 1], BF16)
        nc.gpsimd.memset(ones_sb[:], 1.0)
        v_nat = big.tile([128, NJ, 128], BF16)

        # ---- phase 1: per chunk, fp8 DoubleRow QKV (k + q heads) with
        # fused RoPE, then bf16 v projection + transpose.  The bf16
        # hidden-state chunk for v streams during the fp8 compute of the
        # same chunk (hid buffer is single-buffered: its DMA overlaps the
        # fp8 matmuls, which only touch the fp8 copy). ----
        with (
            tc.tile_pool(name="psk", bufs=2, space="PSUM") as psk,
            tc.tile_pool(name="psT", bufs=2, space="PSUM") as psT,
            tc.tile_pool(name="ropec", bufs=1) as ropec,
            tc.tile_pool(name="ropep", bufs=2) as ropep,
            tc.tile_pool(name="wqp", bufs=1) as wqp,
            tc.tile_pool(name="hfp", bufs=2) as hfp,
            tc.tile_pool(name="wvp", bufs=1, side="right") as wvp,
            tc.tile_pool(name="hidp", bufs=1, side="right") as hidp,
            tc.tile_pool(name="vsp", bufs=2, side="right") as vsp,
        ):
            wqf_t = wqp.tile([128, NF, KO, 128], F8)
            nc.sync.dma_start(wqf_t[:, 0:1], wqf_d.ap()[:, 0:1])
            cos_sb = ropec.tile([128, S], BF16)
            sin_sb = ropec.tile([128, S], BF16)
            ident = ropec.tile([128, 128], BF16)
            nc.sync.dma_start(cos_sb[:], cos_d.ap())
            nc.sync.dma_start(sin_sb[:], sin_d.ap())
            make_identity(nc, ident[:])

            def rope_chunk(et, ci):
                sl = slice(ci * CH, (ci + 1) * CH)
                x = qkv_sb[:, et, sl]
                rot = ropep.tile([128, CH], BF16, tag="rot",
                                 name=f"rot{et}_{ci}")
                nc.scalar.copy(rot[0:64, :], x[64:128, :])
                nc.scalar.copy(rot[64:128, :], x[0:64, :])
                nc.vector.tensor_mul(rot[:], rot[:], sin_sb[:, sl])
                nc.vector.tensor_mul(x, x, cos_sb[:, sl])
                nc.vector.tensor_add(x, x, rot[:])

            wv_t = wvp.tile([128, KO, 128], BF16)
            hf_tiles = {}

            def emit_hf_load(ci):
                hfa = hfp.tile([128, KOH, CH], F8, tag="hfA",
                               name=f"hfA{ci}")
                nc.sync.dma_start(hfa[:], hidf_d.ap()[ci][:, :KOH, :])
                hfb = hfp.tile([128, KOH, CH], F8, tag="hfB",
                               name=f"hfB{ci}")
                nc.sync.dma_start(hfb[:], hidf_d.ap()[ci][:, KOH:, :])
                hf_tiles[ci] = (hfa, hfb)

            emit_hf_load(0)
            for ef in range(1, NF):
                nc.sync.dma_start(wqf_t[:, ef:ef + 1],
                                  wqf_d.ap()[:, ef:ef + 1])
            nc.sync.dma_start(wv_t[:], wqv_d.ap())

            for ci in range(NCH):
                # hid chunk for this ci streams during the fp8 matmuls
                hc = hidp.tile([128, KO, CH], BF16, tag="hc",
                               name=f"hc{ci}")
                nc.sync.dma_start(hc[:, :KOH], hid_d.ap()[ci][:, :KOH, :])
                nc.sync.dma_start(hc[:, KOH:], hid_d.ap()[ci][:, KOH:, :])
                if ci + 1 < NCH:
                    emit_hf_load(ci + 1)
                hf = hf_tiles.pop(ci)
                for ef in range(NF):
                    ps = psk.tile([128, 1024], F32, tag="ps",
                                  name=f"qk{ef}_{ci}")
                    for kp in range(KO // 2):
                        kh, kl = divmod(kp, KPH)
                        w = wqf_t[:, ef, 2 * kp:2 * kp + 2, :]
                        h_t = hf[kh]
                        st, sp = kp == 0, kp == KO // 2 - 1
                        nc.tensor.matmul(
                            ps[:, :CHH], lhsT=w,
                            rhs=h_t[:, 2 * kl:2 * kl + 2, :CHH],
                            perf_mode=mybir.MatmulPerfMode.DoubleRow,
                            start=st, stop=sp)
                        nc.tensor.matmul(
                            ps[:, 512:512 + CHH], lhsT=w,
                            rhs=h_t[:, 2 * kl:2 * kl + 2, CHH:],
                            perf_mode=mybir.MatmulPerfMode.DoubleRow,
                            start=st, stop=sp)
                    nc.vector.tensor_copy(
                        qkv_sb[:, ef, ci * CH:ci * CH + CHH],
                        ps[:, :CHH])
                    nc.vector.tensor_copy(
                        qkv_sb[:, ef, ci * CH + CHH:(ci + 1) * CH],
                        ps[:, 512:512 + CHH])
                    rope_chunk(ef, ci)

                # v projection for this chunk (bf16)
                ps = psk.tile([128, 1024], F32, tag="ps", name=f"vp{ci}")
                for ko in range(KO):
                    st, sp = ko == 0, ko == KO - 1
                    nc.tensor.matmul(
                        ps[:, :CHH], lhsT=wv_t[:, ko, :],
                        rhs=hc[:, ko, :CHH], start=st, stop=sp)
                    nc.tensor.matmul(
                        ps[:, 512:512 + CHH], lhsT=wv_t[:, ko, :],
                        rhs=hc[:, ko, CHH:], start=st, stop=sp)
                vstg = vsp.tile([128, CH], BF16, tag="vs", name=f"vs{ci}")
                nc.vector.tensor_copy(vstg[:, :CHH], ps[:, :CHH])
                nc.vector.tensor_copy(vstg[:, CHH:], ps[:, 512:512 + CHH])
                for sl in range(CH // 128):
                    st_ = ci * (CH // 128) + sl
                    pt = psT.tile([128, 128], BF16, tag="pt",
                                  name=f"tp{st_}")
                    nc.tensor.transpose(
                        pt[:], vstg[:, sl * 128:(sl + 1) * 128], ident[:])
                    nc.vector.tensor_copy(v_nat[:, st_, :], pt[:])

        # w_out pool + oT_full live across attention + out_proj; the first
        # few w_out loads are emitted before the attention instructions so
        # the scalar-engine HWDGE queue streams them during attention.
        with (
            tc.tile_pool(name="wop", bufs=5) as wop,
            tc.tile_pool(name="oTp", bufs=1) as oTp,
        ):
            oT_full = oTp.tile([128, KO2, SB], BF16)
            wo_tiles = {}

            def emit_wo_load(g, nt):
                wo_t = wop.tile([128, GK, OT], BF16, tag="wo",
                                name=f"wo{g}_{nt}")
                nc.scalar.dma_start(
                    wo_t[:], wo_d.ap()[nt][:, g * GK:(g + 1) * GK, :])
                wo_tiles[(g, nt)] = wo_t

            for nt in range(cfg.WOPRE):
                emit_wo_load(0, nt)

            # ---- phase 2+3: attention, normalize per chunk, split A2A.
            # Nothing in this phase ever waits on a collective. ----
            with (
                tc.tile_pool(name="attw", bufs=1, side="right") as attw,
                tc.tile_pool(name="pp", bufs=5, side="right") as pp,
                tc.tile_pool(name="pap", bufs=3, side="right") as pap,
                tc.tile_pool(name="rp", bufs=2, side="right") as rp,
                tc.tile_pool(name="rbp", bufs=2, side="right") as rbp,
                tc.tile_pool(name="stg", bufs=3, side="right") as stg,
                tc.tile_pool(name="pss", bufs=4, space="PSUM") as pss,
                tc.tile_pool(name="psv", bufs=2, space="PSUM") as psv,
                tc.tile_pool(name="psd", bufs=2, space="PSUM") as psd,
            ):
                msk_sb = attw.tile([128, ND, IC], BF16)
                nc.sync.dma_start(msk_sb[:], msk_d.ap())

                kT = qkv_sb[:, ET_K, :]
                for h in range(NQ):
                    qT = qkv_sb[:, 1 + h, :]
                    g, hl = divmod(h, GH)
                    for ci in range(NIC):
                        jt_max = (ci + 1) * ND
                        njp = jt_max // 2
                        pv = psv.tile([128, IC], F32, tag="pv",
                                      name=f"pv{h}_{ci}")
                        dq = psd.tile([1, IC], F32, tag="dq",
                                      name=f"dq{h}_{ci}")
                        p2s = [None] * jt_max

                        def emit_sc(jt):
                            sc = pss.tile([128, IC], F32, tag="sc",
                                          name=f"sc{h}_{ci}_{jt}")
                            nc.tensor.matmul(
                                sc[:],
                                lhsT=kT[:, jt * 128:(jt + 1) * 128],
                                rhs=qT[:, ci * IC:(ci + 1) * IC],
                                start=True, stop=True)
                            p2 = pp.tile([128, IC], BF16, tag="p",
                                         name=f"p{h}_{ci}_{jt}")
                            nc.scalar.activation(
                                p2[:], sc[:],
                                mybir.ActivationFunctionType.Exp,
                                scale=softmax_scale)
                            if jt >= ci * ND:
                                nc.vector.tensor_mul(
                                    p2[:], p2[:],
                                    msk_sb[:, jt - ci * ND, :])
                            p2s[jt] = p2

                        def emit_av(jt):
                            nc.tensor.matmul(
                                pv[:], lhsT=v_nat[:, jt, :], rhs=p2s[jt][:],
                                start=(jt == 0), stop=(jt == jt_max - 1))

                        def emit_pa(jp):
                            # pair-summed P for the softmax denominator:
                            # halves the ones-matmul stream.  All adds on
                            # vector — gpsimd thrashes microcode libraries
                            # switching between tensor ops and broadcasts.
                            a, b = p2s[2 * jp], p2s[2 * jp + 1]
                            pa = pap.tile([128, IC], BF16, tag="pa",
                                          name=f"pa{h}_{ci}_{jp}")
                            nc.vector.tensor_add(pa[:], a[:], b[:])
                            nc.tensor.matmul(
                                dq[:], lhsT=ones_sb[:, 0:1], rhs=pa[:],
                                start=(jp == 0), stop=(jp == njp - 1))

                        emit_sc(0)
                        emit_sc(1)
                        for jt in range(jt_max):
                            if jt + 2 < jt_max:
                                emit_sc(jt + 2)
                            emit_av(jt)
                            if jt % 2 == 1:
                                emit_pa(jt // 2)

                        # chunk epilogue: reciprocal, normalize, ship.
                        # CAST first so the pv PSUM slot frees without
                        # waiting on the reciprocal/broadcast chain.
                        o = stg.tile([128, IC], BF16, tag="o",
                                     name=f"o{h}_{ci}")
                        nc.vector.tensor_copy(o[:], pv[:])
                        r_sb = rp.tile([1, IC], F32, tag="r",
                                       name=f"r{h}_{ci}")
                        nc.vector.reciprocal_approx_fast(r_sb[:], dq[:])
                        rb = rbp.tile([128, IC], F32, tag="rb",
                                      name=f"rb{h}_{ci}")
                        nc.gpsimd.partition_broadcast(rb[:], r_sb[:])
                        nc.vector.tensor_mul(o[:], o[:], rb[:])
                        nc.sync.dma_start(
                            a2a_in[g][2 * ci:2 * ci + 2,
                                      hl * 128:(hl + 1) * 128, :]
                            .rearrange("r p s -> p r s"),
                            o[:].rearrange("p (r s) -> p r s", r=2))
                    if hl == GH - 1:
                        nc.gpsimd.collective_compute(
                            "AllToAll", mybir.AluOpType.bypass,
                            replica_groups=[list(range(cfg.R))],
                            ins=[a2a_in[g][:]], outs=[a2a_out[g][:]])

            # ---- phase 4: out_proj on this core's seq block, three
            # passes of GK=16 k-tiles (one per A2A group), partials
            # stashed in SBUF bf16.  Pass 0/1 run while the last
            # AllToAll may still be in flight on the slowest rank. ----
            with (
                tc.tile_pool(name="ph4", bufs=1) as ph4,
                tc.tile_pool(name="obp", bufs=2) as obp,
                tc.tile_pool(name="psB", bufs=8, space="PSUM") as psB,
            ):
                def emit_gather(g):
                    nc.sync.dma_start(
                        oT_full[:, g * GK:(g + 1) * GK, :]
                        .rearrange("p (r hl) s -> p r hl s", r=cfg.R),
                        a2a_out[g][:]
                        .rearrange("r (hl p) s -> p r hl s", hl=GH))

                for g in range(NG):
                    emit_gather(g)
                stash = ph4.tile([128, NT * 2, OT], BF16)

                for g in range(NG):
                    for nt in range(NT):
                        if (g, nt) not in wo_tiles:
                            emit_wo_load(g, nt)
                        wo_t = wo_tiles[(g, nt)]
                        for mi in range(SB // 128):
                            ps = psB.tile([128, OT], F32, tag="po",
                                          name=f"po{g}_{nt}_{mi}")
                            for k in range(GK):
                                ko = g * GK + k
                                nc.tensor.matmul(
                                    ps[:],
                                    lhsT=oT_full[:, ko,
                                                 mi * 128:(mi + 1) * 128],
                                    rhs=wo_t[:, k, :],
                                    start=(k == 0), stop=(k == GK - 1))
                            sl = stash[:, nt * 2 + mi, :]
                            if g == 0:
                                nc.vector.tensor_copy(sl, ps[:])
                            elif g == 1:
                                nc.vector.tensor_add(sl, sl, ps[:])
                            else:
                                ob = obp.tile([128, OT], F32, tag="ob",
                                              name=f"ob{nt}_{mi}")
                                nc.vector.tensor_add(ob[:], sl, ps[:])
                                nc.sync.dma_start(
                                    out_d.ap()[mi * 128:(mi + 1) * 128,
                                               nt * OT:(nt + 1) * OT],
                                    ob[:])

    nc.compile()
    return nc


def make_masks(cfg: Cfg) -> np.ndarray:
    ND = cfg.IC // 128
    jj = np.arange(128)[:, None, None]
    rr = np.arange(ND)[None, :, None]
    ii = np.arange(cfg.IC)[None, None, :]
    return (jj + 128 * rr <= ii).astype(NP_BF16)


def _to_f8(x):
    return np.ascontiguousarray(
        np.clip(x * F8_SCALE, -F8_CLIP, F8_CLIP)).astype(NP_F8)


def shard_inputs(cfg: Cfg, hidden_states, cos, sin, w_qkv, w_out,
                 n_heads, n_kv):
    """Build per-core input maps (host-side shard + cast + layout)."""
    S, KO, NQ, R = cfg.S, cfg.KO, cfg.NQ, cfg.R
    D = cfg.D
    NCH, CH = S // cfg.CH, cfg.CH
    NF = NQ + 1
    hid_T = np.ascontiguousarray(hidden_states.reshape(S, D).T)  # [D, S]
    # [NCH, 128, KO, CH]
    hid_l = (hid_T.reshape(KO, 128, NCH, CH).transpose(2, 1, 0, 3)
             .astype(NP_BF16))
    hid_l = np.ascontiguousarray(hid_l)
    hidf_l = np.ascontiguousarray(
        _to_f8(hid_T).reshape(KO, 128, NCH, CH).transpose(2, 1, 0, 3))
    NT, OT, KO2 = cfg.NT, cfg.OT, cfg.KO2
    # reorder w_out rows so contraction tile ko2 = g*(R*GH) + r*GH + hl
    # maps to global head r*NQ + g*GH + hl
    NG, GH = cfg.NG, cfg.GH
    w_re = (w_out.reshape(R, NG, GH, 128, cfg.DO).transpose(1, 0, 2, 3, 4)
            .reshape(KO2 * 128, cfg.DO))
    wo_l = (w_re.reshape(KO2, 128, NT, OT).transpose(2, 1, 0, 3)
            .astype(NP_BF16))
    wo_l = np.ascontiguousarray(wo_l)
    cos_T = cos.T.astype(NP_BF16)  # [HD, S]
    sin_T = sin.T
    sinS = np.concatenate([-sin_T[:64], sin_T[64:]], axis=0).astype(NP_BF16)
    masks = make_masks(cfg)

    in_maps = []
    for c in range(R):
        qs = c * NQ * 128
        # fp8 e-tile order: k, q0..q5
        wf = np.concatenate([
            w_qkv[:, n_heads * HD + c * 128: n_heads * HD + (c + 1) * 128],
            w_qkv[:, qs:qs + NQ * 128],
        ], axis=1)  # [D, NF*128]
        wqf_l = (_to_f8(wf).reshape(KO, 128, NF, 128)
                 .transpose(1, 2, 0, 3))  # [128, NF, KO, 128]
        wv = w_qkv[:, (n_heads + n_kv) * HD + c * 128:
                   (n_heads + n_kv) * HD + (c + 1) * 128]  # [D, 128]
        wqv_l = (wv.reshape(KO, 128, 128).transpose(1, 0, 2)
                 .astype(NP_BF16))
        in_maps.append({
            "hid": hid_l, "hidf": hidf_l,
            "wqv": np.ascontiguousarray(wqv_l),
            "wqf": np.ascontiguousarray(wqf_l),
            "wo": wo_l,
            "cosT": cos_T, "sinT": sinS, "masks": masks,
        })
    return in_maps


_cached = {}


def _get_nc(cfg: Cfg):
    key = tuple(sorted(cfg.__dict__.items()))
    if key not in _cached:
        _cached[key] = build(cfg)
    return _cached[key]


def run(cfg: Cfg, in_maps, **kwargs):
    nc = _get_nc(cfg)
    res = run_bass_kernel_spmd(nc, in_maps, core_ids=list(range(cfg.R)),
                               **kwargs)
    out = np.concatenate([res.results[c]["out"] for c in range(cfg.R)],
                         axis=0)
    return out, res


def kernel(hidden_states, cos, sin, w_qkv, w_out):
    cfg = Cfg()
    hidden_states = np.asarray(hidden_states, dtype=np.float32)
    cos = np.asarray(cos, dtype=np.float32)
    sin = np.asarray(sin, dtype=np.float32)
    w_qkv = np.asarray(w_qkv, dtype=np.float32)
    w_out = np.asarray(w_out, dtype=np.float32)
    in_maps = shard_inputs(cfg, hidden_states, cos, sin, w_qkv, w_out, H, KV)
    out, _ = run(cfg, in_maps)
    return out.reshape(B, S, D).astype(np.float32)


# revision 10
# speedup vs baseline: 1.3993x; 1.0196x over previous
"""DbrxAttention (GQA + RoPE + causal) on 8 Trainium2 NeuronCores.

Tensor-parallel over heads: core c owns q heads [6c, 6c+6) and kv head c.
Phase 1 interleaves, per 512-column sequence chunk, the fp8(e4m3)
DoubleRow QKV projection (k + 6 q heads, 2x PE rate, inputs pre-scaled
by 128 host-side; the 2^28 product scale is folded into the softmax
exp) with the bf16 v projection + transpose, so the 25MB bf16
hidden-state stream for v hides under fp8 compute.  Attention is
causal, bf16, scores transposed (kv on partitions, q on free dim),
software-pipelined two j-tiles ahead; the softmax denominator uses
pair-summed P tiles (vector/gpsimd adds) to halve the ones-matmul
stream.  3-way split AllToAll (per head-pair) overlaps attention;
nothing in the attention phase ever waits on a collective.  out_proj
runs as three passes of 16 contraction k-tiles (one per AllToAll
group) with partials stashed in SBUF bf16, so ~200us of tensor work on
already-arrived head groups absorbs cross-core launch skew before the
last group's data is needed.  w_out streams on the scalar-engine HWDGE
queue, prefetched during attention.

kernel(**inputs) takes the full unsharded inputs and returns the full output.
"""

import math

import numpy as np
import ml_dtypes

import concourse.bass as bass
import concourse.mybir as mybir
from concourse import bacc
import concourse.tile as tile
from concourse.bass_utils import run_bass_kernel_spmd
from concourse.masks import make_identity

BF16 = mybir.dt.bfloat16
F32 = mybir.dt.float32
F8 = mybir.dt.float8e4
NP_BF16 = ml_dtypes.bfloat16
NP_F8 = ml_dtypes.float8_e4m3
F8_SCALE = 128.0   # per-operand scale before fp8 cast
F8_CLIP = 224.0    # TRN e4m3 max normal is 240 (inf at 256)

# full-size problem config
B, S, D = 1, 2048, 6144
H, KV, HD = 48, 8, 128
R = 8  # cores


class Cfg:
    def __init__(self, S=2048, KO=48, NQ=6, R=8, DO=6144, IC=512, CH=512,
                 OT=512, GH=2, WOPRE=3):
        self.S = S          # sequence length
        self.KO = KO        # contraction k-tiles for QKV (D = KO*128)
        self.NQ = NQ        # q heads per core
        self.R = R          # cores
        self.DO = DO        # out_proj output dim
        self.IC = IC        # attention i-chunk (free dim per scores matmul)
        self.CH = CH        # QKV s-chunk (pair of CH/2 matmuls)
        self.OT = OT        # out_proj n-chunk
        self.GH = GH        # heads per AllToAll group
        self.WOPRE = WOPRE  # w_out tiles prefetched during attention
        self.D = KO * 128
        self.SB = S // R    # seq block per core after AllToAll
        self.KO2 = R * NQ   # contraction k-tiles for out_proj (H*HD = KO2*128)
        self.NT = DO // OT
        self.NG = NQ // GH  # number of A2A groups
        self.GK = R * GH    # out_proj k-tiles per A2A group
        assert S % R == 0 and S % IC == 0 and S % CH == 0 and DO % OT == 0
        assert IC % 128 == 0
        assert NQ % GH == 0
        assert self.GK * self.NG == self.KO2


def build(cfg: Cfg) -> bacc.Bacc:
    S, KO, NQ, IC, CH = cfg.S, cfg.KO, cfg.NQ, cfg.IC, cfg.CH
    NF = NQ + 1            # fp8 e-tiles per core (k + q heads)
    NCH = S // CH
    NIC = S // IC
    ND = IC // 128         # diagonal j-tiles per i-chunk
    NJ = S // 128
    SB, KO2, NT, DO, OT = cfg.SB, cfg.KO2, cfg.NT, cfg.DO, cfg.OT
    GH, NG, GK = cfg.GH, cfg.NG, cfg.GK
    CHH = CH // 2
    KOH = KO // 2          # ko half for split fp8 DMAs
    KPH = KO // 4          # fp8 DoubleRow k-pairs per ko half
    softmax_scale = (1.0 / math.sqrt(HD)) / (F8_SCALE ** 4)
    ET_K = 0               # e-tile index of k; q head h is e-tile 1+h
    nc = bacc.Bacc("TRN2", target_bir_lowering=False, debug=False,
                   num_devices=cfg.R)

    hid_d = nc.dram_tensor("hid", [NCH, 128, KO, CH], BF16,
                           kind="ExternalInput")
    hidf_d = nc.dram_tensor("hidf", [NCH, 128, KO, CH], F8,
                            kind="ExternalInput")
    wqv_d = nc.dram_tensor("wqv", [128, KO, 128], BF16,
                           kind="ExternalInput")
    wqf_d = nc.dram_tensor("wqf", [128, NF, KO, 128], F8,
                           kind="ExternalInput")
    wo_d = nc.dram_tensor("wo", [NT, 128, KO2, OT], BF16,
                          kind="ExternalInput")
    cos_d = nc.dram_tensor("cosT", [128, S], BF16, kind="ExternalInput")
    sin_d = nc.dram_tensor("sinT", [128, S], BF16, kind="ExternalInput")
    msk_d = nc.dram_tensor("masks", [128, ND, IC], BF16, kind="ExternalInput")
    out_d = nc.dram_tensor("out", [SB, DO], F32, kind="ExternalOutput")

    with (
        tile.TileContext(nc) as tc,
        tc.tile_pool(name="dram", bufs=1, space="DRAM") as dram,
        tc.tile_pool(name="big", bufs=1) as big,
    ):
        a2a_in = [dram.tile([cfg.R, GH * 128, SB], BF16,
                            name=f"a2a_in{g}") for g in range(NG)]
        a2a_out = [dram.tile([cfg.R, GH * 128, SB], BF16,
                             name=f"a2a_out{g}") for g in range(NG)]
        qkv_sb = big.tile([128, NF, S], BF16)
        ones_sb = big.tile([128, 1], BF16)
        nc.gpsimd.memset(ones_sb[:], 1.0)
        ones_row = big.tile([1, 128], BF16)
        nc.gpsimd.memset(ones_row[:], 1.0)
        v_nat = big.tile([128, NJ, 128], BF16)

        # ---- phase 1: per chunk, fp8 DoubleRow QKV (k + q heads) with
        # fused RoPE, then bf16 v projection + transpose.  The bf16
        # hidden-state chunk for v streams during the fp8 compute of the
        # same chunk (hid buffer is single-buffered: its DMA overlaps the
        # fp8 matmuls, which only touch the fp8 copy). ----
        with (
            tc.tile_pool(name="psk", bufs=2, space="PSUM") as psk,
            tc.tile_pool(name="psT", bufs=2, space="PSUM") as psT,
            tc.tile_pool(name="ropec", bufs=1) as ropec,
            tc.tile_pool(name="ropep", bufs=2) as ropep,
            tc.tile_pool(name="wqp", bufs=1) as wqp,
            tc.tile_pool(name="hfp", bufs=2) as hfp,
            tc.tile_pool(name="wvp", bufs=1, side="right") as wvp,
            tc.tile_pool(name="hidp", bufs=1, side="right") as hidp,
            tc.tile_pool(name="vsp", bufs=2, side="right") as vsp,
        ):
            wqf_t = wqp.tile([128, NF, KO, 128], F8)
            nc.sync.dma_start(wqf_t[:, 0:1], wqf_d.ap()[:, 0:1])
            cos_sb = ropec.tile([128, S], BF16)
            sin_sb = ropec.tile([128, S], BF16)
            ident = ropec.tile([128, 128], BF16)
            nc.sync.dma_start(cos_sb[:], cos_d.ap())
            nc.sync.dma_start(sin_sb[:], sin_d.ap())
            make_identity(nc, ident[:])

            def rope_chunk(et, ci):
                sl = slice(ci * CH, (ci + 1) * CH)
                x = qkv_sb[:, et, sl]
                rot = ropep.tile([128, CH], BF16, tag="rot",
                                 name=f"rot{et}_{ci}")
                nc.scalar.copy(rot[0:64, :], x[64:128, :])
                nc.scalar.copy(rot[64:128, :], x[0:64, :])
                nc.vector.tensor_mul(rot[:], rot[:], sin_sb[:, sl])
                nc.vector.tensor_mul(x, x, cos_sb[:, sl])
                nc.vector.tensor_add(x, x, rot[:])

            wv_t = wvp.tile([128, KO, 128], BF16)
            hf_tiles = {}

            def emit_hf_load(ci):
                hfa = hfp.tile([128, KOH, CH], F8, tag="hfA",
                               name=f"hfA{ci}")
                nc.sync.dma_start(hfa[:], hidf_d.ap()[ci][:, :KOH, :])
                hfb = hfp.tile([128, KOH, CH], F8, tag="hfB",
                               name=f"hfB{ci}")
                nc.sync.dma_start(hfb[:], hidf_d.ap()[ci][:, KOH:, :])
                hf_tiles[ci] = (hfa, hfb)

            emit_hf_load(0)
            for ef in range(1, NF):
                nc.sync.dma_start(wqf_t[:, ef:ef + 1],
                                  wqf_d.ap()[:, ef:ef + 1])
            nc.sync.dma_start(wv_t[:], wqv_d.ap())

            for ci in range(NCH):
                # hid chunk for this ci streams during the fp8 matmuls
                hc = hidp.tile([128, KO, CH], BF16, tag="hc",
                               name=f"hc{ci}")
                nc.sync.dma_start(hc[:, :KOH], hid_d.ap()[ci][:, :KOH, :])
                nc.sync.dma_start(hc[:, KOH:], hid_d.ap()[ci][:, KOH:, :])
                if ci + 1 < NCH:
                    emit_hf_load(ci + 1)
                hf = hf_tiles.pop(ci)
                for ef in range(NF):
                    ps = psk.tile([128, 1024], F32, tag="ps",
                                  name=f"qk{ef}_{ci}")
                    for kp in range(KO // 2):
                        kh, kl = divmod(kp, KPH)
                        w = wqf_t[:, ef, 2 * kp:2 * kp + 2, :]
                        h_t = hf[kh]
                        st, sp = kp == 0, kp == KO // 2 - 1
                        nc.tensor.matmul(
                            ps[:, :CHH], lhsT=w,
                            rhs=h_t[:, 2 * kl:2 * kl + 2, :CHH],
                            perf_mode=mybir.MatmulPerfMode.DoubleRow,
                            start=st, stop=sp)
                        nc.tensor.matmul(
                            ps[:, 512:512 + CHH], lhsT=w,
                            rhs=h_t[:, 2 * kl:2 * kl + 2, CHH:],
                            perf_mode=mybir.MatmulPerfMode.DoubleRow,
                            start=st, stop=sp)
                    nc.vector.tensor_copy(
                        qkv_sb[:, ef, ci * CH:ci * CH + CHH],
                        ps[:, :CHH])
                    nc.vector.tensor_copy(
                        qkv_sb[:, ef, ci * CH + CHH:(ci + 1) * CH],
                        ps[:, 512:512 + CHH])
                    rope_chunk(ef, ci)

                # v projection for this chunk (bf16)
                ps = psk.tile([128, 1024], F32, tag="ps", name=f"vp{ci}")
                for ko in range(KO):
                    st, sp = ko == 0, ko == KO - 1
                    nc.tensor.matmul(
                        ps[:, :CHH], lhsT=wv_t[:, ko, :],
                        rhs=hc[:, ko, :CHH], start=st, stop=sp)
                    nc.tensor.matmul(
                        ps[:, 512:512 + CHH], lhsT=wv_t[:, ko, :],
                        rhs=hc[:, ko, CHH:], start=st, stop=sp)
                vstg = vsp.tile([128, CH], BF16, tag="vs", name=f"vs{ci}")
                nc.vector.tensor_copy(vstg[:, :CHH], ps[:, :CHH])
                nc.vector.tensor_copy(vstg[:, CHH:], ps[:, 512:512 + CHH])
                for sl in range(CH // 128):
                    st_ = ci * (CH // 128) + sl
                    pt = psT.tile([128, 128], BF16, tag="pt",
                                  name=f"tp{st_}")
                    nc.tensor.transpose(
                        pt[:], vstg[:, sl * 128:(sl + 1) * 128], ident[:])
                    nc.vector.tensor_copy(v_nat[:, st_, :], pt[:])

        # w_out pool + oT_full live across attention + out_proj; the first
        # few w_out loads are emitted before the attention instructions so
        # the scalar-engine HWDGE queue streams them during attention.
        with (
            tc.tile_pool(name="wop", bufs=5) as wop,
            tc.tile_pool(name="oTp", bufs=1) as oTp,
        ):
            oT_full = oTp.tile([128, KO2, SB], BF16)
            wo_tiles = {}

            def emit_wo_load(g, nt):
                wo_t = wop.tile([128, GK, OT], BF16, tag="wo",
                                name=f"wo{g}_{nt}")
                nc.scalar.dma_start(
                    wo_t[:], wo_d.ap()[nt][:, g * GK:(g + 1) * GK, :])
                wo_tiles[(g, nt)] = wo_t

            for nt in range(cfg.WOPRE):
                emit_wo_load(0, nt)

            # ---- phase 2+3: attention, normalize per chunk, split A2A.
            # Nothing in this phase ever waits on a collective. ----
            with (
                tc.tile_pool(name="attw", bufs=1, side="right") as attw,
                tc.tile_pool(name="pp", bufs=4, side="right") as pp,
                tc.tile_pool(name="pap", bufs=3, side="right") as pap,
                tc.tile_pool(name="rp", bufs=2, side="right") as rp,
                tc.tile_pool(name="stg", bufs=3, side="right") as stg,
                tc.tile_pool(name="pss", bufs=2, space="PSUM") as pss,
                tc.tile_pool(name="psv", bufs=2, space="PSUM") as psv,
                tc.tile_pool(name="psd", bufs=1, space="PSUM") as psd,
                tc.tile_pool(name="psr", bufs=1, space="PSUM") as psr,
            ):
                msk_sb = attw.tile([128, ND, IC], BF16)
                nc.sync.dma_start(msk_sb[:], msk_d.ap())

                kT = qkv_sb[:, ET_K, :]
                for h in range(NQ):
                    qT = qkv_sb[:, 1 + h, :]
                    g, hl = divmod(h, GH)
                    for ci in range(NIC):
                        jt_max = (ci + 1) * ND
                        njp = jt_max // 2
                        pv = psv.tile([128, IC], F32, tag="pv",
                                      name=f"pv{h}_{ci}")
                        dq = psd.tile([1, IC], F32, tag="dq",
                                      name=f"dq{h}_{ci}")
                        p2s = [None] * njp

                        def emit_sc(jp):
                            # pair of score j-tiles in one PSUM tile
                            # so the exp runs at full 1024 width.
                            sc = pss.tile([128, 1024], F32, tag="sc",
                                          name=f"sc{h}_{ci}_{jp}")
                            for u in range(2):
                                jt = 2 * jp + u
                                nc.tensor.matmul(
                                    sc[:, u * 512:u * 512 + IC],
                                    lhsT=kT[:, jt * 128:(jt + 1) * 128],
                                    rhs=qT[:, ci * IC:(ci + 1) * IC],
                                    start=True, stop=True)
                            p2 = pp.tile([128, 1024], BF16, tag="p",
                                         name=f"p{h}_{ci}_{jp}")
                            nc.scalar.activation(
                                p2[:], sc[:],
                                mybir.ActivationFunctionType.Exp,
                                scale=softmax_scale)
                            for u in range(2):
                                jt = 2 * jp + u
                                if jt >= ci * ND:
                                    nc.vector.tensor_mul(
                                        p2[:, u * 512:u * 512 + IC],
                                        p2[:, u * 512:u * 512 + IC],
                                        msk_sb[:, jt - ci * ND, :])
                            p2s[jp] = p2

                        def emit_av(jp):
                            p2 = p2s[jp]
                            for u in range(2):
                                jt = 2 * jp + u
                                nc.tensor.matmul(
                                    pv[:], lhsT=v_nat[:, jt, :],
                                    rhs=p2[:, u * 512:u * 512 + IC],
                                    start=(jt == 0),
                                    stop=(jt == jt_max - 1))
                            # pair-summed P for the softmax denominator:
                            # halves the ones-matmul stream
                            pa = pap.tile([128, IC], BF16, tag="pa",
                                          name=f"pa{h}_{ci}_{jp}")
                            nc.vector.tensor_add(pa[:], p2[:, :IC],
                                                 p2[:, 512:512 + IC])
                            nc.tensor.matmul(
                                dq[:], lhsT=ones_sb[:, 0:1], rhs=pa[:],
                                start=(jp == 0), stop=(jp == njp - 1))

                        emit_sc(0)
                        if njp > 1:
                            emit_sc(1)
                        for jp in range(njp):
                            if jp + 2 < njp:
                                emit_sc(jp + 2)
                            emit_av(jp)

                        # chunk epilogue: reciprocal, broadcast via a K=1
                        # ones-row matmul (gpsimd partition_broadcast
                        # thrashes microcode libs), normalize, ship.
                        o = stg.tile([128, IC], BF16, tag="o",
                                     name=f"o{h}_{ci}")
                        nc.vector.tensor_copy(o[:], pv[:])
                        r_sb = rp.tile([1, IC], F32, tag="r",
                                       name=f"r{h}_{ci}")
                        nc.vector.reciprocal_approx_fast(r_sb[:], dq[:])
                        r_bf = rp.tile([1, IC], BF16, tag="rbf",
                                       name=f"rbf{h}_{ci}")
                        nc.scalar.copy(r_bf[:], r_sb[:])
                        rb = psr.tile([128, IC], F32, tag="rb",
                                      name=f"rb{h}_{ci}")
                        nc.tensor.matmul(rb[:], lhsT=ones_row[0:1, :],
                                         rhs=r_bf[0:1, :],
                                         start=True, stop=True)
                        nc.vector.tensor_mul(o[:], o[:], rb[:])
                        nc.sync.dma_start(
                            a2a_in[g][2 * ci:2 * ci + 2,
                                      hl * 128:(hl + 1) * 128, :]
                            .rearrange("r p s -> p r s"),
                            o[:].rearrange("p (r s) -> p r s", r=2))
                    if hl == GH - 1:
                        nc.gpsimd.collective_compute(
                            "AllToAll", mybir.AluOpType.bypass,
                            replica_groups=[list(range(cfg.R))],
                            ins=[a2a_in[g][:]], outs=[a2a_out[g][:]])

            # ---- phase 4: out_proj on this core's seq block, three
            # passes of GK=16 k-tiles (one per A2A group), partials
            # stashed in SBUF bf16.  Pass 0/1 run while the last
            # AllToAll may still be in flight on the slowest rank. ----
            with (
                tc.tile_pool(name="ph4", bufs=1) as ph4,
                tc.tile_pool(name="obp", bufs=2) as obp,
                tc.tile_pool(name="psB", bufs=8, space="PSUM") as psB,
            ):
                def emit_gather(g):
                    nc.sync.dma_start(
                        oT_full[:, g * GK:(g + 1) * GK, :]
                        .rearrange("p (r hl) s -> p r hl s", r=cfg.R),
                        a2a_out[g][:]
                        .rearrange("r (hl p) s -> p r hl s", hl=GH))

                # floor the gathers' scheduler timestamp so they land in
                # the sync stream after all attention ships — otherwise
                # the scheduler hoists them between ships and their wait
                # on the AllToAll stalls the whole ship queue.
                with tc.tile_wait_until(0.45):
                    for g in range(NG):
                        emit_gather(g)
                stash = ph4.tile([128, NT * 2, OT], BF16)

                for g in range(NG):
                    for nt in range(NT):
                        if (g, nt) not in wo_tiles:
                            emit_wo_load(g, nt)
                        wo_t = wo_tiles[(g, nt)]
                        for mi in range(SB // 128):
                            ps = psB.tile([128, OT], F32, tag="po",
                                          name=f"po{g}_{nt}_{mi}")
                            for k in range(GK):
                                ko = g * GK + k
                                nc.tensor.matmul(
                                    ps[:],
                                    lhsT=oT_full[:, ko,
                                                 mi * 128:(mi + 1) * 128],
                                    rhs=wo_t[:, k, :],
                                    start=(k == 0), stop=(k == GK - 1))
                            sl = stash[:, nt * 2 + mi, :]
                            if g == 0:
                                nc.vector.tensor_copy(sl, ps[:])
                            elif g == 1:
                                nc.vector.tensor_add(sl, sl, ps[:])
                            else:
                                ob = obp.tile([128, OT], F32, tag="ob",
                                              name=f"ob{nt}_{mi}")
                                nc.vector.tensor_add(ob[:], sl, ps[:])
                                nc.sync.dma_start(
                                    out_d.ap()[mi * 128:(mi + 1) * 128,
                                               nt * OT:(nt + 1) * OT],
                                    ob[:])

    nc.compile()
    return nc


def make_masks(cfg: Cfg) -> np.ndarray:
    ND = cfg.IC // 128
    jj = np.arange(128)[:, None, None]
    rr = np.arange(ND)[None, :, None]
    ii = np.arange(cfg.IC)[None, None, :]
    return (jj + 128 * rr <= ii).astype(NP_BF16)


def _to_f8(x):
    return np.ascontiguousarray(
        np.clip(x * F8_SCALE, -F8_CLIP, F8_CLIP)).astype(NP_F8)


def shard_inputs(cfg: Cfg, hidden_states, cos, sin, w_qkv, w_out,
                 n_heads, n_kv):
    """Build per-core input maps (host-side shard + cast + layout)."""
    S, KO, NQ, R = cfg.S, cfg.KO, cfg.NQ, cfg.R
    D = cfg.D
    NCH, CH = S // cfg.CH, cfg.CH
    NF = NQ + 1
    hid_T = np.ascontiguousarray(hidden_states.reshape(S, D).T)  # [D, S]
    # [NCH, 128, KO, CH]
    hid_l = (hid_T.reshape(KO, 128, NCH, CH).transpose(2, 1, 0, 3)
             .astype(NP_BF16))
    hid_l = np.ascontiguousarray(hid_l)
    hidf_l = np.ascontiguousarray(
        _to_f8(hid_T).reshape(KO, 128, NCH, CH).transpose(2, 1, 0, 3))
    NT, OT, KO2 = cfg.NT, cfg.OT, cfg.KO2
    # reorder w_out rows so contraction tile ko2 = g*(R*GH) + r*GH + hl
    # maps to global head r*NQ + g*GH + hl
    NG, GH = cfg.NG, cfg.GH
    w_re = (w_out.reshape(R, NG, GH, 128, cfg.DO).transpose(1, 0, 2, 3, 4)
            .reshape(KO2 * 128, cfg.DO))
    wo_l = (w_re.reshape(KO2, 128, NT, OT).transpose(2, 1, 0, 3)
            .astype(NP_BF16))
    wo_l = np.ascontiguousarray(wo_l)
    cos_T = cos.T.astype(NP_BF16)  # [HD, S]
    sin_T = sin.T
    sinS = np.concatenate([-sin_T[:64], sin_T[64:]], axis=0).astype(NP_BF16)
    masks = make_masks(cfg)

    in_maps = []
    for c in range(R):
        qs = c * NQ * 128
        # fp8 e-tile order: k, q0..q5
        wf = np.concatenate([
            w_qkv[:, n_heads * HD + c * 128: n_heads * HD + (c + 1) * 128],
            w_qkv[:, qs:qs + NQ * 128],
        ], axis=1)  # [D, NF*128]
        wqf_l = (_to_f8(wf).reshape(KO, 128, NF, 128)
                 .transpose(1, 2, 0, 3))  # [128, NF, KO, 128]
        wv = w_qkv[:, (n_heads + n_kv) * HD + c * 128:
                   (n_heads + n_kv) * HD + (c + 1) * 128]  # [D, 128]
        wqv_l = (wv.reshape(KO, 128, 128).transpose(1, 0, 2)
                 .astype(NP_BF16))
        in_maps.append({
            "hid": hid_l, "hidf": hidf_l,
            "wqv": np.ascontiguousarray(wqv_l),
            "wqf": np.ascontiguousarray(wqf_l),
            "wo": wo_l,
            "cosT": cos_T, "sinT": sinS, "masks": masks,
        })
    return in_maps


_cached = {}


def _get_nc(cfg: Cfg):
    key = tuple(sorted(cfg.__dict__.items()))
    if key not in _cached:
        _cached[key] = build(cfg)
    return _cached[key]


def run(cfg: Cfg, in_maps, **kwargs):
    nc = _get_nc(cfg)
    res = run_bass_kernel_spmd(nc, in_maps, core_ids=list(range(cfg.R)),
                               **kwargs)
    out = np.concatenate([res.results[c]["out"] for c in range(cfg.R)],
                         axis=0)
    return out, res


def kernel(hidden_states, cos, sin, w_qkv, w_out):
    cfg = Cfg()
    hidden_states = np.asarray(hidden_states, dtype=np.float32)
    cos = np.asarray(cos, dtype=np.float32)
    sin = np.asarray(sin, dtype=np.float32)
    w_qkv = np.asarray(w_qkv, dtype=np.float32)
    w_out = np.asarray(w_out, dtype=np.float32)
    in_maps = shard_inputs(cfg, hidden_states, cos, sin, w_qkv, w_out, H, KV)
    out, _ = run(cfg, in_maps)
    return out.reshape(B, S, D).astype(np.float32)
